# revision 2
# baseline (speedup 1.0000x reference)
"""Trainium2 Bass kernel for nn_ButterflyFactorNewMlp.

Computes: attn = einsum('ds,td->st', w1, w2) * sparse_mask
          out  = gelu(einsum('bds,st->bdt', x, attn) + b2)   (exact erf gelu)

Key structural fact (verified against the reference mask): mask[s,t] != 0
iff  s//81 == t//81  and  (s%27)//3 == (t%27)//3.  Writing
s = 81A + 27B + 3C + D, the condition is A_s==A_t and C_s==C_t — so under
the permutation s -> (A, C, B, D) the masked attn becomes block-diagonal
with 81 DENSE 9x9 blocks (6561 nonzeros total).

Sharding (chosen over the data-parallel hint): shard the OUTPUT feature
axis t across the 8 cores.  Core c owns 91-92 consecutive permuted
t-columns; those columns only read the 99 permuted s-rows of the 11
groups they straddle.  Every core therefore:
  - receives x^T pre-permuted/transposed on host: [100, 49152] fp16
    (99 feature rows + a ones row that carries the bias),
  - computes its own [99, 92] attn patch from the w1/w2 column slices
    (23 accumulating fp16 matmuls over the 2916-dim hidden contraction,
    then a DVE mask-multiply; b2 is cast-DMA'd into lhsT row 99),
  - streams all 49152 tokens through ONE stationary-weight matmul per
    512-token chunk (attn patch is the PE-stationary operand), exact-erf
    gelu straight out of PSUM, fp16 out^T stores.

Per-core traffic is ~20MB (x^T 9.8 + out^T 9.0 + weights 1.1) vs ~27MB
for batch-sharding, PE work drops ~4x (no transposes, no dense-K waste),
and there are no collectives.  Host does the (cheap) permute/transpose
on both ends; device time is DMA-bound near the 358 GB/s roofline.

Precision: fp16 inputs/weights, fp32 PSUM accumulation, erf-gelu LUT on
the fp32 accumulator, fp16 stores — end-to-end ~7e-4 relative error.
"""

import sys

if "/opt/trn_rl_repo" not in sys.path:
    sys.path.insert(0, "/opt/trn_rl_repo")

import numpy as np

import concourse.bacc as bacc
import concourse.mybir as mybir
import concourse.tile as tile
from concourse.bass import ds
from concourse.bass_utils import run_bass_kernel_spmd

F32 = mybir.dt.float32
F16 = mybir.dt.float16
GELU = mybir.ActivationFunctionType.Gelu

N_CORES = 8
B, D, S = 64, 768, 729          # batch, channels, features (729 = in = out)
H = 2916                        # hidden dim of the weight contraction
HP = 2944                       # hidden padded to 23*128
N_KD = HP // 128                # 23 contraction chunks for the attn matmuls
M = B * D                       # 49152 tokens (shared by every core)
KR = 99                         # s-rows per core (11 groups x 9)
K_IN = KR + 1                   # + ones row for the bias
W = 92                          # t-columns per core, padded
CHUNK = 512                     # tokens per matmul (one PSUM bank)
MACRO = 4096                    # tokens per DMA transfer
N_MACRO = M // MACRO            # 12
SUB = MACRO // CHUNK            # 8

# permuted t-column boundaries per core (92 cols for core 3, 91 otherwise);
# every core's columns straddle exactly 11 of the 81 groups -> 99 s-rows
BOUNDS = [0, 91, 182, 273, 365, 456, 547, 638, 729]

_COMPILED = None
LAST = None  # BassKernelResults of the most recent kernel() call (for test.py)


def _perm():
    s = np.arange(S)
    key = (s // 81) * 81 + ((s % 27) // 3) * 9 + ((s % 81) // 27) * 3 + (s % 3)
    return np.argsort(key, kind="stable")


def _build():
    nc = bacc.Bacc("TRN2", target_bir_lowering=False, debug=False)

    x_d = nc.dram_tensor("x", [K_IN, M], F16, kind="ExternalInput")
    w1_d = nc.dram_tensor("w1s", [HP, KR], F16, kind="ExternalInput")
    w2_d = nc.dram_tensor("w2ts", [HP, W], F16, kind="ExternalInput")
    mask_d = nc.dram_tensor("maskc", [KR, W], F16, kind="ExternalInput")
    b2_d = nc.dram_tensor("b2c", [1, W], F32, kind="ExternalInput")
    out_d = nc.dram_tensor("out", [W, M], F16, kind="ExternalOutput")

    with tile.TileContext(nc) as tc:
        with (
            tc.tile_pool(name="const", bufs=1) as cpool,
            tc.tile_pool(name="xin", bufs=3) as xpool,
            tc.tile_pool(name="oout", bufs=3) as opool,
            tc.tile_pool(name="ps1", bufs=1, space="PSUM") as ps1pool,
            tc.tile_pool(name="ps2", bufs=5, space="PSUM") as ps2pool,
        ):
            # ------- stage 1: this core's [99, 92] attn patch -------
            w1_sb = cpool.tile([128, N_KD, KR], F16)
            w2_sb = cpool.tile([128, N_KD, W], F16)
            nc.sync.dma_start(
                w1_sb[:], w1_d[:].rearrange("(c p) f -> p c f", p=128)
            )
            nc.sync.dma_start(
                w2_sb[:], w2_d[:].rearrange("(c p) f -> p c f", p=128)
            )
            mask_sb = cpool.tile([128, W], F16)
            nc.sync.dma_start(mask_sb[0:KR, :], mask_d[:])

            attn_sb = cpool.tile([128, W], F16)
            ps1 = ps1pool.tile([128, W], F32)
            for kd in range(N_KD):
                nc.tensor.matmul(
                    ps1[0:KR, :],
                    w1_sb[:, kd, :],
                    w2_sb[:, kd, :],
                    start=(kd == 0),
                    stop=(kd == N_KD - 1),
                )
            nc.vector.tensor_tensor(
                attn_sb[0:KR, :], ps1[0:KR, :], mask_sb[0:KR, :],
                mybir.AluOpType.mult,
            )
            # bias row rides the contraction via the x ones-row (cast f32->f16)
            nc.gpsimd.dma_start(attn_sb[KR:K_IN, :], b2_d[:])

            # ------- stage 2: stream all tokens through the patch -------
            for mi in range(N_MACRO):
                x_sb = xpool.tile([128, MACRO], F16, tag="x")
                nc.sync.dma_start(
                    x_sb[0:K_IN, :], x_d[:, ds(mi * MACRO, MACRO)]
                )
                o_sb = opool.tile([128, MACRO], F16, tag="o")
                for si in range(SUB):
                    ps = ps2pool.tile([128, CHUNK], F32, tag="ps")
                    nc.tensor.matmul(
                        ps[0:W, :],
                        attn_sb[0:K_IN, :],
                        x_sb[0:K_IN, ds(si * CHUNK, CHUNK)],
                        start=True,
                        stop=True,
                    )
                    nc.scalar.activation(
                        o_sb[0:W, ds(si * CHUNK, CHUNK)], ps[0:W, :], GELU
                    )
                nc.sync.dma_start(
                    out_d[:, ds(mi * MACRO, MACRO)], o_sb[0:W, :]
                )

    nc.compile()
    return nc


def kernel(x, w1, w2, b2, sparse_mask):
    global _COMPILED, LAST
    if _COMPILED is None:
        _COMPILED = _build()
    nc = _COMPILED

    x = np.asarray(x, dtype=np.float32)
    w1 = np.asarray(w1, dtype=np.float32)
    w2 = np.asarray(w2, dtype=np.float32)
    b2 = np.asarray(b2, dtype=np.float32)
    mask = np.asarray(sparse_mask, dtype=np.float32)

    perm = _perm()
    xh = x.reshape(M, S).astype(np.float16)
    xTp = xh.T[perm]                       # [729, 49152] permuted x^T
    w1p = w1[:, perm]
    w2p = w2[perm, :]
    maskp = mask[np.ix_(perm, perm)]
    b2p = b2[perm]

    in_maps = []
    for c in range(N_CORES):
        t0, t1 = BOUNDS[c], BOUNDS[c + 1]
        wid = t1 - t0
        s0 = 9 * (t0 // 9)
        xc = np.empty((K_IN, M), np.float16)
        xc[0:KR] = xTp[s0 : s0 + KR]
        xc[KR] = np.float16(1.0)
        w1c = np.zeros((HP, KR), np.float16)
        w1c[:H] = w1p[:, s0 : s0 + KR]
        w2c = np.zeros((HP, W), np.float16)
        w2c[:H, 0:wid] = w2p[t0:t1].T
        mc = np.zeros((KR, W), np.float16)
        mc[:, 0:wid] = maskp[s0 : s0 + KR, t0:t1]
        bc = np.zeros((1, W), np.float32)
        bc[0, 0:wid] = b2p[t0:t1]
        in_maps.append(
            {"x": xc, "w1s": w1c, "w2ts": w2c, "maskc": mc, "b2c": bc}
        )

    LAST = run_bass_kernel_spmd(nc, in_maps, list(range(N_CORES)))

    outp = np.empty((S, M), np.float16)
    for c in range(N_CORES):
        t0, t1 = BOUNDS[c], BOUNDS[c + 1]
        outp[t0:t1] = LAST.results[c]["out"][0 : t1 - t0]
    final = np.empty((M, S), np.float32)
    final[:, perm] = outp.T
    return final.reshape(B, D, S)


# revision 10
# speedup vs baseline: 1.0481x; 1.0481x over previous
"""Trainium2 Bass kernel for nn_ButterflyFactorNewMlp.

Computes: attn = einsum('ds,td->st', w1, w2) * sparse_mask
          out  = gelu(einsum('bds,st->bdt', x, attn) + b2)   (exact erf gelu)

Key structural fact (verified against the reference mask): mask[s,t] != 0
iff  s//81 == t//81  and  (s%27)//3 == (t%27)//3.  Writing
s = 81A + 27B + 3C + D, the condition is A_s==A_t and C_s==C_t — so under
the permutation s -> (A, C, B, D) the masked attn becomes block-diagonal
with 81 DENSE 9x9 blocks (6561 nonzeros total).

Sharding (chosen over the data-parallel hint): shard the OUTPUT feature
axis t across the 8 cores.  Core c owns 91-92 consecutive permuted
t-columns; those only read the 99 permuted s-rows of the 11 groups they
straddle.  Every core:
  - receives x^T pre-permuted/transposed on host: [100, 49152] fp16
    (99 feature rows + a ones row that carries the bias),
  - computes its own [99, 92] attn patch from w1/w2 column slices
    (23 accumulating fp16 matmuls over the 2916 hidden dims, DVE
    mask-multiply; b2 cast-DMA'd into lhsT row 99),
  - streams all 49152 tokens through ONE stationary-weight matmul per
    512-token chunk (the attn patch stays PE-stationary), erf-gelu out
    of PSUM in 2048-wide ACTIVATEs, fp16 out^T stores.

Per-core traffic ~20MB (x^T 9.8 + out^T 9.0 + weights 1.1) vs ~27MB for
batch-sharding; no collectives; PE work ~4x below the batch-sharded
formulation.  Host does the cheap permute/transpose on both ends.

Trace-driven DMA shaping: SDMA engine parallelism follows descriptor
count (~32 descriptors per engine slot), so every bulk transfer is
expressed as [p, 8, 512]-style 3D APs -> 1KB descriptors, >=512 per
DMA, spreading each stream across all 16 engines.  The x-in stream
issues on the sync HWDGE ring; weights + gelu + out-stores issue on the
scalar ring so the two streams never serialize each other's issue.

Precision: fp16 inputs/weights, fp32 PSUM accumulation, erf-gelu LUT on
the fp32 accumulator, fp16 stores — end-to-end ~7e-4 relative error.
"""

import sys

if "/opt/trn_rl_repo" not in sys.path:
    sys.path.insert(0, "/opt/trn_rl_repo")

import numpy as np

import concourse.bacc as bacc
import concourse.mybir as mybir
import concourse.tile as tile
from concourse.bass import ds
from concourse.bass_utils import run_bass_kernel_spmd

F32 = mybir.dt.float32
F16 = mybir.dt.float16
GELU = mybir.ActivationFunctionType.Gelu

N_CORES = 8
B, D, S = 64, 768, 729          # batch, channels, features (729 = in = out)
H = 2916                        # hidden dim of the weight contraction
HP = 2944                       # hidden padded to 23*128
N_KD = HP // 128                # 23 contraction chunks for the attn matmuls
M = B * D                       # 49152 tokens (shared by every core)
KR = 99                         # s-rows per core (11 groups x 9)
K_IN = KR + 1                   # + ones row for the bias
W = 92                          # t-columns per core, padded
CHUNK = 512                     # tokens per matmul (one PSUM bank)
MACRO = 4096                    # tokens per DMA transfer
N_MACRO = M // MACRO            # 12
SUB = MACRO // CHUNK            # 8 matmuls / 2 activations per macro
GRP = 2048                      # tokens per PSUM tile / ACTIVATE call

# permuted t-column boundaries per core (92 cols for core 3, 91 otherwise);
# every core's columns straddle exactly 11 of the 81 groups -> 99 s-rows
BOUNDS = [0, 91, 182, 273, 365, 456, 547, 638, 729]

_COMPILED = None
LAST = None  # BassKernelResults of the most recent kernel() call (for test.py)


def _perm():
    s = np.arange(S)
    key = (s // 81) * 81 + ((s % 27) // 3) * 9 + ((s % 81) // 27) * 3 + (s % 3)
    return np.argsort(key, kind="stable")


def _build():
    nc = bacc.Bacc("TRN2", target_bir_lowering=False, debug=False)

    # 3D shapes so plain slices yield many ~1KB descriptors per DMA
    x_d = nc.dram_tensor("x", [K_IN, M // CHUNK, CHUNK], F16, kind="ExternalInput")
    # weights pre-packed on host into the SBUF-resident layout
    w1_d = nc.dram_tensor("w1s", [128, N_KD, KR], F16, kind="ExternalInput")
    w2_d = nc.dram_tensor("w2ts", [128, N_KD, W], F16, kind="ExternalInput")
    mask_d = nc.dram_tensor("maskc", [KR, W], F16, kind="ExternalInput")
    b2_d = nc.dram_tensor("b2c", [1, W], F32, kind="ExternalInput")
    out_d = nc.dram_tensor("out", [W, M // CHUNK, CHUNK], F16, kind="ExternalOutput")

    with tile.TileContext(nc) as tc:
        with (
            tc.tile_pool(name="const", bufs=1) as cpool,
            tc.tile_pool(name="xin", bufs=4) as xpool,
            tc.tile_pool(name="oout", bufs=3) as opool,
            tc.tile_pool(name="ps", bufs=2, space="PSUM") as pspool,
        ):
            # ------- stage 1: this core's [99, 92] attn patch -------
            # weights ride the scalar (Activation) HWDGE ring; x-in owns sync
            w1_sb = cpool.tile([128, N_KD, KR], F16)
            w2_sb = cpool.tile([128, N_KD, W], F16)
            nc.scalar.dma_start(w1_sb[:], w1_d[:])
            nc.scalar.dma_start(w2_sb[:], w2_d[:])
            mask_sb = cpool.tile([128, W], F16)
            nc.scalar.dma_start(mask_sb[0:KR, :], mask_d[:])

            attn_sb = cpool.tile([128, W], F16)
            ps1 = pspool.tile([128, 4, CHUNK], F32, tag="ps", name="ps1")
            for kd in range(N_KD):
                nc.tensor.matmul(
                    ps1[0:KR, 0, 0:W],
                    w1_sb[:, kd, :],
                    w2_sb[:, kd, :],
                    start=(kd == 0),
                    stop=(kd == N_KD - 1),
                )
            nc.vector.tensor_tensor(
                attn_sb[0:KR, :], ps1[0:KR, 0, 0:W], mask_sb[0:KR, :],
                mybir.AluOpType.mult,
            )
            # bias row rides the contraction via the x ones-row (cast f32->f16)
            nc.gpsimd.dma_start(attn_sb[KR:K_IN, :], b2_d[:])

            # ------- stage 2: stream all tokens through the patch -------
            for mi in range(N_MACRO):
                # [100, 8, 512] -> 800 descriptors of 1KB: all 16 SDMA engines
                x_sb = xpool.tile([128, SUB, CHUNK], F16, tag="x")
                nc.sync.dma_start(
                    x_sb[0:K_IN, :, :], x_d[:, ds(mi * SUB, SUB), :]
                )
                o_sb = opool.tile([128, SUB, CHUNK], F16, tag="o")
                for half in range(MACRO // GRP):
                    ps = pspool.tile([128, 4, CHUNK], F32, tag="ps", name="ps2")
                    for q in range(GRP // CHUNK):
                        si = half * (GRP // CHUNK) + q
                        nc.tensor.matmul(
                            ps[0:W, q, :],
                            attn_sb[0:K_IN, :],
                            x_sb[0:K_IN, si, :],
                            start=True,
                            stop=True,
                        )
                    # one wide ACTIVATE per 2048 tokens (4 PSUM banks)
                    nc.scalar.activation(
                        o_sb[0:W, ds(half * 4, 4), :], ps[0:W, :, :], GELU
                    )
                nc.scalar.dma_start(
                    out_d[:, ds(mi * SUB, SUB), :], o_sb[0:W, :, :]
                )

    nc.compile()
    return nc


def _pack_weights(wc):
    """[HP, F] -> [128, N_KD, F]: partition-major SBUF-resident layout."""
    hp, f = wc.shape
    return np.ascontiguousarray(wc.reshape(N_KD, 128, f).transpose(1, 0, 2))


def kernel(x, w1, w2, b2, sparse_mask):
    global _COMPILED, LAST
    if _COMPILED is None:
        _COMPILED = _build()
    nc = _COMPILED

    x = np.asarray(x, dtype=np.float32)
    w1 = np.asarray(w1, dtype=np.float32)
    w2 = np.asarray(w2, dtype=np.float32)
    b2 = np.asarray(b2, dtype=np.float32)
    mask = np.asarray(sparse_mask, dtype=np.float32)

    perm = _perm()
    xh = x.reshape(M, S).astype(np.float16)
    xTp = xh.T[perm]                       # [729, 49152] permuted x^T
    w1p = w1[:, perm]
    w2p = w2[perm, :]
    maskp = mask[np.ix_(perm, perm)]
    b2p = b2[perm]

    in_maps = []
    for c in range(N_CORES):
        t0, t1 = BOUNDS[c], BOUNDS[c + 1]
        wid = t1 - t0
        s0 = 9 * (t0 // 9)
        xc = np.empty((K_IN, M), np.float16)
        xc[0:KR] = xTp[s0 : s0 + KR]
        xc[KR] = np.float16(1.0)
        w1c = np.zeros((HP, KR), np.float16)
        w1c[:H] = w1p[:, s0 : s0 + KR]
        w2c = np.zeros((HP, W), np.float16)
        w2c[:H, 0:wid] = w2p[t0:t1].T
        mc = np.zeros((KR, W), np.float16)
        mc[:, 0:wid] = maskp[s0 : s0 + KR, t0:t1]
        bc = np.zeros((1, W), np.float32)
        bc[0, 0:wid] = b2p[t0:t1]
        in_maps.append(
            {
                "x": xc.reshape(K_IN, M // CHUNK, CHUNK),
                "w1s": _pack_weights(w1c),
                "w2ts": _pack_weights(w2c),
                "maskc": mc,
                "b2c": bc,
            }
        )

    LAST = run_bass_kernel_spmd(nc, in_maps, list(range(N_CORES)))

    outp = np.empty((S, M), np.float16)
    for c in range(N_CORES):
        t0, t1 = BOUNDS[c], BOUNDS[c + 1]
        outp[t0:t1] = LAST.results[c]["out"].reshape(W, M)[0 : t1 - t0]
    final = np.empty((M, S), np.float32)
    final[:, perm] = outp.T
    return final.reshape(B, D, S)


# revision 11
# speedup vs baseline: 1.5628x; 1.4911x over previous
"""Trainium2 Bass kernel for nn_ButterflyFactorNewMlp.

Computes: attn = einsum('ds,td->st', w1, w2) * sparse_mask
          out  = gelu(einsum('bds,st->bdt', x, attn) + b2)   (exact erf gelu)

Key structural fact (verified against the reference mask): mask[s,t] != 0
iff  s//81 == t//81  and  (s%27)//3 == (t%27)//3.  Writing
s = 81A + 27B + 3C + D, the condition is A_s==A_t and C_s==C_t — so under
the permutation s -> (A, C, B, D) the masked attn becomes block-diagonal
with 81 DENSE 9x9 blocks (6561 nonzeros total).

Sharding (chosen over the data-parallel hint): shard the OUTPUT feature
axis t across the 8 cores.  Core c owns 91-92 consecutive permuted
t-columns; those only read the 99 permuted s-rows of the 11 groups they
straddle.  Every core:
  - receives x^T pre-permuted/transposed on host: [99, 49152] fp16,
  - computes its own [99, 92] attn patch from w1/w2 column slices
    (23 accumulating fp16 matmuls over the 2916 hidden dims + DVE
    mask-multiply),
  - streams all 49152 tokens through ONE stationary-weight matmul per
    512-token chunk (the attn patch stays PE-stationary), erf-gelu out
    of PSUM in 2048-wide ACTIVATEs with b2 applied via the per-partition
    bias port, fp16 out^T stores.

Per-core traffic ~21MB (x^T 9.6 + out^T 9.0 + weights 2.3) with no
collectives; PE work ~4x below the batch-sharded formulation.  Host does
the cheap permute/transpose on both ends.

Trace-driven DMA shaping (measured on HW): a transfer's packets
round-robin over C SDMA engines where C = the largest divisor of its
partition count that is <= 16.  All bulk transfers are therefore issued
with partition counts 96/128 (C=16) or 80+12 splits — a 100-partition
transfer gets C=10 and a 92-partition one C=4, which single-handedly
capped v1/v2 at ~110GB/s.  The x-in stream issues on the sync HWDGE
ring; weights + out-stores issue on the scalar ring so the two streams
never serialize each other's issue.  Each DMA uses [p, 8, 512]-style 3D
APs (1KB descriptors -> 8KB per-partition packets).

Precision: fp16 inputs/weights, fp32 PSUM accumulation, erf-gelu LUT on
the fp32 accumulator, fp16 stores — end-to-end ~7e-4 relative error.
"""

import sys

if "/opt/trn_rl_repo" not in sys.path:
    sys.path.insert(0, "/opt/trn_rl_repo")

import numpy as np

import concourse.bacc as bacc
import concourse.mybir as mybir
import concourse.tile as tile
from concourse.bass import ds
from concourse.bass_utils import run_bass_kernel_spmd

F32 = mybir.dt.float32
F16 = mybir.dt.float16
GELU = mybir.ActivationFunctionType.Gelu

N_CORES = 8
B, D, S = 64, 768, 729          # batch, channels, features (729 = in = out)
H = 2916                        # hidden dim of the weight contraction
HP = 2944                       # hidden padded to 23*128
N_KD = HP // 128                # 23 contraction chunks for the attn matmuls
M = B * D                       # 49152 tokens (shared by every core)
KR = 99                         # s-rows per core (11 groups x 9)
KSPL = 96                       # x-in partition split: 96 (C=16) + 3 (C=3)
W = 92                          # t-columns per core, padded
WSPL = 80                       # out-store partition split: 80 (C=16) + 12 (C=12)
CHUNK = 512                     # tokens per matmul (one PSUM bank)
MACRO = 4096                    # tokens per DMA transfer
N_MACRO = M // MACRO            # 12
SUB = MACRO // CHUNK            # 8 matmuls / 2 activations per macro
GRP = 2048                      # tokens per PSUM tile / ACTIVATE call

# permuted t-column boundaries per core (92 cols for core 3, 91 otherwise);
# every core's columns straddle exactly 11 of the 81 groups -> 99 s-rows
BOUNDS = [0, 91, 182, 273, 365, 456, 547, 638, 729]

_COMPILED = None
LAST = None  # BassKernelResults of the most recent kernel() call (for test.py)


def _perm():
    s = np.arange(S)
    key = (s // 81) * 81 + ((s % 27) // 3) * 9 + ((s % 81) // 27) * 3 + (s % 3)
    return np.argsort(key, kind="stable")


def _build():
    nc = bacc.Bacc("TRN2", target_bir_lowering=False, debug=False)

    # 3D shapes so plain slices yield many ~1KB descriptors per DMA
    x_d = nc.dram_tensor("x", [KR, M // CHUNK, CHUNK], F16, kind="ExternalInput")
    # weights pre-packed on host into the SBUF-resident layout
    w1_d = nc.dram_tensor("w1s", [128, N_KD, KR], F16, kind="ExternalInput")
    w2_d = nc.dram_tensor("w2ts", [128, N_KD, W], F16, kind="ExternalInput")
    mask_d = nc.dram_tensor("maskc", [KR, W], F16, kind="ExternalInput")
    b2_d = nc.dram_tensor("b2c", [W, 1], F32, kind="ExternalInput")
    out_d = nc.dram_tensor("out", [W, M // CHUNK, CHUNK], F16, kind="ExternalOutput")

    with tile.TileContext(nc) as tc:
        with (
            tc.tile_pool(name="const", bufs=1) as cpool,
            tc.tile_pool(name="xin", bufs=4) as xpool,
            tc.tile_pool(name="oout", bufs=3) as opool,
            tc.tile_pool(name="ps", bufs=2, space="PSUM") as pspool,
        ):
            # ------- stage 1: this core's [99, 92] attn patch -------
            # weights ride the scalar (Activation) HWDGE ring; x-in owns sync
            w1_sb = cpool.tile([128, N_KD, KR], F16)
            w2_sb = cpool.tile([128, N_KD, W], F16)
            nc.scalar.dma_start(w1_sb[:], w1_d[:])
            nc.scalar.dma_start(w2_sb[:], w2_d[:])
            mask_sb = cpool.tile([128, W], F16)
            nc.sync.dma_start(mask_sb[0:KR, :], mask_d[:])
            b2_sb = cpool.tile([128, 1], F32)
            nc.sync.dma_start(b2_sb[0:W, :], b2_d[:])

            attn_sb = cpool.tile([128, W], F16)
            ps1 = pspool.tile([128, 4, CHUNK], F32, tag="ps", name="ps1")
            for kd in range(N_KD):
                nc.tensor.matmul(
                    ps1[0:KR, 0, 0:W],
                    w1_sb[:, kd, :],
                    w2_sb[:, kd, :],
                    start=(kd == 0),
                    stop=(kd == N_KD - 1),
                )
            nc.vector.tensor_tensor(
                attn_sb[0:KR, :], ps1[0:KR, 0, 0:W], mask_sb[0:KR, :],
                mybir.AluOpType.mult,
            )

            # ------- stage 2: stream all tokens through the patch -------
            for mi in range(N_MACRO):
                # partition-count splits keep every DMA on 16 (resp 12)
                # SDMA engines; [p, 8, 512] APs -> 8KB per-partition packets
                x_sb = xpool.tile([128, SUB, CHUNK], F16, tag="x")
                nc.sync.dma_start(
                    x_sb[0:KSPL, :, :], x_d[0:KSPL, ds(mi * SUB, SUB), :]
                )
                nc.sync.dma_start(
                    x_sb[KSPL:KR, :, :], x_d[KSPL:KR, ds(mi * SUB, SUB), :]
                )
                o_sb = opool.tile([128, SUB, CHUNK], F16, tag="o")
                for half in range(MACRO // GRP):
                    ps = pspool.tile([128, 4, CHUNK], F32, tag="ps", name="ps2")
                    for q in range(GRP // CHUNK):
                        si = half * (GRP // CHUNK) + q
                        nc.tensor.matmul(
                            ps[0:W, q, :],
                            attn_sb[0:KR, :],
                            x_sb[0:KR, si, :],
                            start=True,
                            stop=True,
                        )
                    # one wide ACTIVATE per 2048 tokens (4 PSUM banks);
                    # b2 rides the per-partition bias port: gelu(x + b2)
                    nc.scalar.activation(
                        o_sb[0:W, ds(half * 4, 4), :],
                        ps[0:W, :, :],
                        GELU,
                        bias=b2_sb[0:W, :],
                    )
                nc.scalar.dma_start(
                    out_d[0:WSPL, ds(mi * SUB, SUB), :], o_sb[0:WSPL, :, :]
                )
                nc.scalar.dma_start(
                    out_d[WSPL:W, ds(mi * SUB, SUB), :], o_sb[WSPL:W, :, :]
                )

    nc.compile()
    return nc


def _pack_weights(wc):
    """[HP, F] -> [128, N_KD, F]: partition-major SBUF-resident layout."""
    hp, f = wc.shape
    return np.ascontiguousarray(wc.reshape(N_KD, 128, f).transpose(1, 0, 2))


def kernel(x, w1, w2, b2, sparse_mask):
    global _COMPILED, LAST
    if _COMPILED is None:
        _COMPILED = _build()
    nc = _COMPILED

    x = np.asarray(x, dtype=np.float32)
    w1 = np.asarray(w1, dtype=np.float32)
    w2 = np.asarray(w2, dtype=np.float32)
    b2 = np.asarray(b2, dtype=np.float32)
    mask = np.asarray(sparse_mask, dtype=np.float32)

    perm = _perm()
    xh = x.reshape(M, S).astype(np.float16)
    xTp = xh.T[perm]                       # [729, 49152] permuted x^T
    w1p = w1[:, perm]
    w2p = w2[perm, :]
    maskp = mask[np.ix_(perm, perm)]
    b2p = b2[perm]

    in_maps = []
    for c in range(N_CORES):
        t0, t1 = BOUNDS[c], BOUNDS[c + 1]
        wid = t1 - t0
        s0 = 9 * (t0 // 9)
        xc = np.ascontiguousarray(xTp[s0 : s0 + KR])
        w1c = np.zeros((HP, KR), np.float16)
        w1c[:H] = w1p[:, s0 : s0 + KR]
        w2c = np.zeros((HP, W), np.float16)
        w2c[:H, 0:wid] = w2p[t0:t1].T
        mc = np.zeros((KR, W), np.float16)
        mc[:, 0:wid] = maskp[s0 : s0 + KR, t0:t1]
        bc = np.zeros((W, 1), np.float32)
        bc[0:wid, 0] = b2p[t0:t1]
        in_maps.append(
            {
                "x": xc.reshape(KR, M // CHUNK, CHUNK),
                "w1s": _pack_weights(w1c),
                "w2ts": _pack_weights(w2c),
                "maskc": mc,
                "b2c": bc,
            }
        )

    LAST = run_bass_kernel_spmd(nc, in_maps, list(range(N_CORES)))

    outp = np.empty((S, M), np.float16)
    for c in range(N_CORES):
        t0, t1 = BOUNDS[c], BOUNDS[c + 1]
        outp[t0:t1] = LAST.results[c]["out"].reshape(W, M)[0 : t1 - t0]
    final = np.empty((M, S), np.float32)
    final[:, perm] = outp.T
    return final.reshape(B, D, S)
